# revision 16
# baseline (speedup 1.0000x reference)
"""
MultiHeadLatentMoE layer as a Bass/Tile kernel for 8 Trainium2 NeuronCores.

Problem (T=8192, D=1024, NH=8 heads, DH=128, NE=8 experts/head, top-2, DHID=512):
    h      = (x @ in_w.T + in_b).reshape(T, NH, DH)
    logits = einsum('tnd,ned->tne', h, router_w)            (fp32)
    gate   = scatter(softmax(top2(logits)))                  (T, NH, NE)
    hid    = gelu(einsum('tnd,nefd->tnef', h, w_in))         (exact erf gelu)
    ye     = einsum('tnef,nefd->tned', hid, w_out)
    y      = einsum('tne,tned->tnd', gate, ye)
    out    = y.reshape(T, NH*DH) @ out_w.T + out_b

Strategy (v2, sparse): data-parallel over tokens (1024/core, zero collectives),
but unlike the dense baseline only the top-2 of 8 experts are computed:

  1. in-proj + router run in token-on-partitions layout.  Router logits come
     from a host-folded matrix C = in_w_head.T @ router_w (exact fp32), so
     routing matches the fp32 reference while h itself is a single fp32r pass.
  2. top-2 masks -> per-(head,expert) positions via a triangular-matmul cumsum
     -> global slot ids (capacity 384/expert, 9 sigma above the 256 mean).
  3. dispatch: indirect DMA scatter of h rows (bf16) into a slot-major DRAM
     table; per head the table is read back transposed (xbar DMA) so the
     expert FFNs run dense bf16 matmuls over 3072 slots instead of 8192.
  4. gates are NOT applied in the expert phase; the combine step indirect-
     gathers each token's two ye rows and blends with w1/w2 on DVE.
  5. out-proj in bf16.

All big matmuls are bf16/fp32r at 1 cycle/row; expert FLOPs drop 2.7x.
"""

import sys

for _p in ("/opt/trn_rl_repo", "/root/.axon_site/_ro/trn_rl_repo"):
    if _p not in sys.path:
        sys.path.append(_p)

import numpy as np
import ml_dtypes

import concourse.bass as bass
import concourse.mybir as mybir
import concourse.tile as tile
from concourse import bacc
from concourse.bass_utils import run_bass_kernel_spmd
from concourse.masks import make_identity

T, D, NH, DH, NE, TOPK, DHID = 8192, 1024, 8, 128, 8, 2, 512
NCORES = 8
TLOC = T // NCORES          # 1024 tokens per core
P = 128
KT = D // P                 # 8 contraction k-tiles for D=1024
NT = TLOC // P              # 8 token tiles of 128
FT = DHID // P              # 4 dhid tiles per expert
CAP = 384                   # token capacity per (head, expert); mean 256
SPH = NE * CAP              # slots per head (3072)
NSLOT = NH * SPH            # total slots (24576)
ST = CAP // P               # slot tiles per expert (3)
F32 = mybir.dt.float32
F32R = mybir.dt.float32r
BF16 = mybir.dt.bfloat16
I32 = mybir.dt.int32
BFNP = ml_dtypes.bfloat16

_CACHED = None
TRACE = False          # set True (e.g. from test.py) to neuron-profile the run
LAST_RESULT = None     # BassKernelResults of the most recent kernel() call
SIM = False            # True: full-table indirect APs (CoreSim-compatible);
                       # False: HW contract (indirect AP rows == #indices)
DEBUG = False          # True: add debug ExternalOutputs for intermediates


def build_program():
    nc = bacc.Bacc()

    xt_d = nc.dram_tensor("xt", [D, TLOC], F32R, kind="ExternalInput")
    inwt_d = nc.dram_tensor("inwt", [D, D], F32R, kind="ExternalInput")
    cfold_d = nc.dram_tensor("cfold", [D, NH * NE], F32, kind="ExternalInput")
    lb_d = nc.dram_tensor("lb", [NH * NE], F32, kind="ExternalInput")
    inb_d = nc.dram_tensor("inb", [D], F32, kind="ExternalInput")
    outb_d = nc.dram_tensor("outb", [D], F32, kind="ExternalInput")
    geoff_d = nc.dram_tensor("geoff", [NH * NE], F32, kind="ExternalInput")
    u_d = nc.dram_tensor("ustrict", [P, P], F32, kind="ExternalInput")
    ones_d = nc.dram_tensor("onesm", [P, P], F32, kind="ExternalInput")
    w_int_d = nc.dram_tensor("w_int", [NH, NE, DH, DHID], BF16, kind="ExternalInput")
    w_outt_d = nc.dram_tensor("w_outt", [NH, NE, DHID, DH], BF16, kind="ExternalInput")
    outwt_d = nc.dram_tensor("outwt", [D, D], BF16, kind="ExternalInput")
    hgt_ds = [nc.dram_tensor(f"hgt{n}", [SPH, DH], BF16) for n in range(NH)]
    yet_ds = [nc.dram_tensor(f"yet{n}", [SPH, DH], BF16) for n in range(NH)]
    out_d = nc.dram_tensor("out_t", [D, TLOC], F32, kind="ExternalOutput")
    if DEBUG:
        dbg_slot1 = nc.dram_tensor("dbg_slot1", [P, NT, NH], I32, kind="ExternalOutput")
        dbg_slot2 = nc.dram_tensor("dbg_slot2", [P, NT, NH], I32, kind="ExternalOutput")
        dbg_w1 = nc.dram_tensor("dbg_w1", [P, NT, NH], F32, kind="ExternalOutput")
        dbg_w2 = nc.dram_tensor("dbg_w2", [P, NT, NH], F32, kind="ExternalOutput")
        dbg_h = nc.dram_tensor("dbg_h", [NT, P, D], BF16, kind="ExternalOutput")
        dbg_hg = nc.dram_tensor("dbg_hg", [P, SPH], BF16, kind="ExternalOutput")
        dbg_hgt0 = nc.dram_tensor("dbg_hgt0", [SPH, DH], BF16, kind="ExternalOutput")
        dbg_yet0 = nc.dram_tensor("dbg_yet0", [SPH, DH], BF16, kind="ExternalOutput")
        dbg_y = nc.dram_tensor("dbg_y", [P, NH, TLOC], BF16, kind="ExternalOutput")

    Act = mybir.ActivationFunctionType
    Alu = mybir.AluOpType
    X = mybir.AxisListType.X

    with tile.TileContext(nc) as tc:
        with (
            tc.tile_pool(name="persist", bufs=1) as persist,
            tc.tile_pool(name="work", bufs=2) as work,
            tc.tile_pool(name="psum", bufs=1, space="PSUM") as psum,
        ):
            # ---- persistent SBUF ----
            ident = persist.tile([P, P], F32, tag="ident")
            make_identity(nc, ident)
            xt_sb = persist.tile([P, KT, TLOC], F32R, tag="xt")
            nc.sync.dma_start(xt_sb[:], xt_d[:].rearrange("(kt p) t -> p kt t", p=P))
            inwt_sb = persist.tile([P, KT, D], F32R, tag="inwt")
            nc.sync.dma_start(
                inwt_sb[:], inwt_d[:].rearrange("(kt p) d -> p kt d", p=P))
            c_sb = persist.tile([P, KT, NH * NE], F32, tag="cfold")
            nc.sync.dma_start(
                c_sb[:], cfold_d[:].rearrange("(kt p) ne -> p kt ne", p=P))
            u_sb = persist.tile([P, P], F32, tag="u")
            nc.sync.dma_start(u_sb[:], u_d[:])
            ones_sb = persist.tile([P, P], F32, tag="ones")
            nc.sync.dma_start(ones_sb[:], ones_d[:])
            inb_bc = persist.tile([P, D], F32, tag="inb")
            nc.sync.dma_start(inb_bc[:], inb_d[None, :].to_broadcast([P, D]))
            lb_bc = persist.tile([P, NH * NE], F32, tag="lb")
            nc.sync.dma_start(lb_bc[:], lb_d[None, :].to_broadcast([P, NH * NE]))
            geoff_bc = persist.tile([P, NH * NE], F32, tag="geoff")
            nc.sync.dma_start(
                geoff_bc[:], geoff_d[None, :].to_broadcast([P, NH * NE]))
            outb_sb = persist.tile([P, KT], F32, tag="outb")
            nc.sync.dma_start(outb_sb[:], outb_d[:].rearrange("(m p) -> p m", p=P))

            slot1_all = persist.tile([P, NT, NH], I32, tag="s1")
            slot2_all = persist.tile([P, NT, NH], I32, tag="s2")
            w1_all = persist.tile([P, NT, NH], F32, tag="w1")
            w2_all = persist.tile([P, NT, NH], F32, tag="w2")
            msum = persist.tile([P, NH * NE], F32, tag="msum")
            nc.vector.memset(msum[:], 0.0)
            y_sb = persist.tile([P, NH, TLOC], BF16, tag="y")

            # ============ Phase 1: in-proj + router + slots (t-par) ============
            for tk in range(NT):
                tsl = slice(tk * P, (tk + 1) * P)
                h_bf = work.tile([P, D], BF16, tag="hbf")
                for hb in range(2):
                    h_ps = psum.tile([P, 512], F32, tag="misc", bufs=2)
                    csl = slice(hb * 512, (hb + 1) * 512)
                    for kt in range(KT):
                        nc.tensor.matmul(
                            h_ps[:],
                            lhsT=xt_sb[:, kt, tsl],
                            rhs=inwt_sb[:, kt, csl],
                            start=(kt == 0),
                            stop=(kt == KT - 1),
                        )
                    # h = psum + in_b  (free-axis bias), cast to bf16
                    nc.vector.tensor_tensor(
                        h_bf[:, csl], h_ps[:], inb_bc[:, csl], Alu.add)

                lg_ps = psum.tile([P, NH * NE], F32, tag="misc", bufs=2)
                for kt in range(KT):
                    nc.tensor.matmul(
                        lg_ps[:],
                        lhsT=xt_sb[:, kt, tsl].bitcast(F32),
                        rhs=c_sb[:, kt, :],
                        start=(kt == 0),
                        stop=(kt == KT - 1),
                    )
                lg = work.tile([P, NH, NE], F32, tag="lg")
                nc.vector.tensor_tensor(
                    lg[:], lg_ps[:].rearrange("p (n e) -> p n e", n=NH),
                    lb_bc[:].rearrange("p (n e) -> p n e", n=NH), Alu.add)

                # top-2 selection per head
                m1 = work.tile([P, NH], F32, tag="m1")
                nc.vector.tensor_reduce(m1[:], lg[:], X, Alu.max)
                eq1 = work.tile([P, NH, NE], F32, tag="eq1")
                nc.vector.tensor_tensor(
                    eq1[:], lg[:], m1[:, :, None].to_broadcast([P, NH, NE]),
                    Alu.is_equal)
                msk = work.tile([P, NH, NE], F32, tag="msk")
                nc.vector.scalar_tensor_tensor(
                    msk[:], eq1[:], -1e30, lg[:], Alu.mult, Alu.add)
                m2 = work.tile([P, NH], F32, tag="m2")
                nc.vector.tensor_reduce(m2[:], msk[:], X, Alu.max)
                eq2 = work.tile([P, NH, NE], F32, tag="eq2")
                nc.vector.tensor_tensor(
                    eq2[:], lg[:], m2[:, :, None].to_broadcast([P, NH, NE]),
                    Alu.is_equal)
                dm = work.tile([P, NH], F32, tag="dm")
                nc.vector.tensor_sub(dm[:], m2[:], m1[:])
                nc.scalar.activation(w2_all[:, tk, :], dm[:], Act.Sigmoid)
                nc.vector.tensor_scalar(
                    w1_all[:, tk, :], w2_all[:, tk, :], -1.0, 1.0,
                    Alu.mult, Alu.add)

                # slot ids: pos = (#selected before t) via triangular matmul
                mask = work.tile([P, NH * NE], F32, tag="mask")
                nc.vector.tensor_tensor(
                    mask[:].rearrange("p (n e) -> p n e", n=NH), eq1[:], eq2[:],
                    Alu.add)
                pos_ps = psum.tile([P, NH * NE], F32, tag="misc", bufs=2)
                nc.tensor.matmul(
                    pos_ps[:], lhsT=u_sb[:], rhs=mask[:], start=True, stop=False)
                nc.tensor.matmul(
                    pos_ps[:], lhsT=ones_sb[:], rhs=msum[:], start=False, stop=True)
                nc.vector.tensor_tensor(msum[:], msum[:], mask[:], Alu.add)
                # clamp to capacity (overflow at 9 sigma: last slot reused)
                posc = work.tile([P, NH * NE], F32, tag="posc")
                nc.vector.tensor_scalar_min(posc[:], pos_ps[:], float(CAP - 1))
                slotv = work.tile([P, NH * NE], F32, tag="slotv")
                nc.vector.tensor_tensor(slotv[:], posc[:], geoff_bc[:], Alu.add)
                for sl_all, eq in ((slot1_all, eq1), (slot2_all, eq2)):
                    t1 = work.tile([P, NH, NE], F32, tag="t1")
                    nc.vector.tensor_tensor(
                        t1[:], eq[:],
                        slotv[:].rearrange("p (n e) -> p n e", n=NH), Alu.mult)
                    s1f = work.tile([P, NH], F32, tag="s1f")
                    nc.vector.tensor_reduce(s1f[:], t1[:], X, Alu.add)
                    # exact-tie safety: a non-one-hot eq sums two slot ids;
                    # clamp so indices stay inside the per-head table
                    nc.vector.tensor_scalar_min(s1f[:], s1f[:], float(SPH - 1))
                    nc.vector.tensor_copy(sl_all[:, tk, :], s1f[:])

                # dispatch: scatter h rows (bf16) to slot-major table.
                # HW contract: indirect-side AP row count == #indices (128),
                # idx is a per-partition column; index*rowstride addresses the
                # full table regardless of the AP slice.
                if DEBUG:
                    nc.sync.dma_start(dbg_h[tk], h_bf[:])
                for sl_all in (slot1_all, slot2_all):
                    for n in range(NH):
                        hrows = hgt_ds[n][:] if SIM else hgt_ds[n][:P, :]
                        nc.gpsimd.indirect_dma_start(
                            out=hrows,
                            out_offset=bass.IndirectOffsetOnAxis(
                                ap=sl_all[:, tk, n:n + 1], axis=0),
                            in_=h_bf[:, n * DH:(n + 1) * DH],
                            in_offset=None,
                        )

            if DEBUG:
                nc.sync.dma_start(dbg_slot1[:], slot1_all[:])
                nc.sync.dma_start(dbg_slot2[:], slot2_all[:])
                nc.sync.dma_start(dbg_w1[:], w1_all[:])
                nc.sync.dma_start(dbg_w2[:], w2_all[:])

            # ============ Phase 2: experts (sparse slots, bf16) ============
            with tc.tile_pool(name="hpool", bufs=2) as hpool, \
                 tc.tile_pool(name="epool", bufs=3) as epool:
                for n in range(NH):
                    hg = hpool.tile([P, SPH], BF16, tag="hg")
                    nc.sync.dma_start_transpose(hg[:], hgt_ds[n][:])
                    # whole head's expert outputs staged in SBUF, written to
                    # DRAM as ONE full-tensor DMA: the combine gather's
                    # nominal read region (rows 0..127) is then strictly
                    # inside this write region, giving correct ordering.
                    yet_all = hpool.tile([P, NE * ST, DH], BF16, tag="yetall")
                    if DEBUG and n == 0:
                        nc.sync.dma_start(dbg_hg[:], hg[:])
                        nc.sync.dma_start(dbg_hgt0[:], hgt_ds[0][:])
                    for e in range(NE):
                        wi = epool.tile([P, DHID], BF16, tag="wi")
                        nc.sync.dma_start(wi[:], w_int_d[n, e])
                        wo = epool.tile([P, FT, DH], BF16, tag="wo")
                        nc.sync.dma_start(
                            wo[:],
                            w_outt_d[n, e].rearrange("(f p) d -> p f d", p=P))
                        esl = slice(e * CAP, (e + 1) * CAP)
                        hidg = epool.tile([P, FT, CAP], BF16, tag="hidg")
                        for f in range(FT):
                            hid_ps = psum.tile(
                                [P, 512], F32, tag=f"hid{f}", bufs=1)
                            nc.tensor.matmul(
                                hid_ps[:, :CAP],
                                lhsT=wi[:, f * P:(f + 1) * P],
                                rhs=hg[:, esl],
                                start=True, stop=True,
                            )
                            nc.scalar.activation(
                                hidg[:, f, :], hid_ps[:, :CAP], Act.Gelu)
                        for st in range(ST):
                            yet_ps = psum.tile([P, DH], F32, tag="misc", bufs=2)
                            ssl = slice(st * P, (st + 1) * P)
                            for f in range(FT):
                                nc.tensor.matmul(
                                    yet_ps[:],
                                    lhsT=hidg[:, f, ssl],
                                    rhs=wo[:, f, :],
                                    start=(f == 0),
                                    stop=(f == FT - 1),
                                )
                            nc.vector.tensor_copy(
                                yet_all[:, e * ST + st, :], yet_ps[:])
                    nc.sync.dma_start(
                        yet_ds[n][:].rearrange("(j p) d -> p j d", p=P),
                        yet_all[:])

            if DEBUG:
                nc.sync.dma_start(dbg_yet0[:], yet_ds[0][:])

            # ============ Phase 3: combine (gather + blend + transpose) ========
            for tk in range(NT):
                tsl = slice(tk * P, (tk + 1) * P)
                y1 = work.tile([P, NH, DH], BF16, tag="y1")
                y2 = work.tile([P, NH, DH], BF16, tag="y2")
                for yt, sl_all in ((y1, slot1_all), (y2, slot2_all)):
                    for n in range(NH):
                        yrows = yet_ds[n][:] if SIM else yet_ds[n][:P, :]
                        nc.gpsimd.indirect_dma_start(
                            out=yt[:, n, :],
                            out_offset=None,
                            in_=yrows,
                            in_offset=bass.IndirectOffsetOnAxis(
                                ap=sl_all[:, tk, n:n + 1], axis=0),
                        )
                ya = work.tile([P, NH, DH], F32, tag="ya")
                nc.vector.tensor_tensor(
                    ya[:], y1[:],
                    w1_all[:, tk, :, None].to_broadcast([P, NH, DH]), Alu.mult)
                yb = work.tile([P, NH, DH], F32, tag="yb")
                nc.vector.tensor_tensor(
                    yb[:], y2[:],
                    w2_all[:, tk, :, None].to_broadcast([P, NH, DH]), Alu.mult)
                yc = work.tile([P, NH, DH], F32, tag="yc")
                nc.vector.tensor_tensor(yc[:], ya[:], yb[:], Alu.add)
                for n in range(NH):
                    tp_ps = psum.tile([P, P], F32, tag="misc", bufs=2)
                    nc.tensor.transpose(tp_ps[:], yc[:, n, :], ident[:])
                    nc.vector.tensor_copy(y_sb[:, n, tsl], tp_ps[:])

            if DEBUG:
                nc.sync.dma_start(dbg_y[:], y_sb[:])

            # ============ Phase 4: out-projection (bf16) ============
            with tc.tile_pool(name="opool", bufs=2) as opool:
                for m in range(KT):
                    ow = opool.tile([P, KT, P], BF16, tag="ow")
                    nc.sync.dma_start(
                        ow[:],
                        outwt_d[:, m * P:(m + 1) * P].rearrange(
                            "(kt p) d -> p kt d", p=P))
                    o_sb = opool.tile([P, TLOC], F32, tag="osb")
                    for tt in range(2):
                        tsl = slice(tt * 512, (tt + 1) * 512)
                        o_ps = psum.tile([P, 512], F32, tag="misc", bufs=2)
                        for kt in range(KT):
                            nc.tensor.matmul(
                                o_ps[:],
                                lhsT=ow[:, kt, :],
                                rhs=y_sb[:, kt, tsl],
                                start=(kt == 0),
                                stop=(kt == KT - 1),
                            )
                        nc.scalar.activation(
                            o_sb[:, tsl], o_ps[:], Act.Identity,
                            bias=outb_sb[:, m:m + 1])
                    nc.sync.dma_start(out_d[m * P:(m + 1) * P, :], o_sb[:])

    nc.compile()
    return nc


def _prep(x, in_w, in_b, router_w, w_in, w_out, out_w, out_b):
    """Host-side layout prep: transposes, bf16 casts, folded router."""
    x = np.ascontiguousarray(x, dtype=np.float32)
    inwt = np.ascontiguousarray(in_w.T, dtype=np.float32)            # (D, D)
    iw = np.asarray(in_w, np.float64).reshape(NH, DH, D)
    rw = np.asarray(router_w, np.float64)                            # (NH,NE,DH)
    cfold = np.einsum("nhd,neh->dne", iw, rw).reshape(D, NH * NE)
    lb = np.einsum(
        "nh,neh->ne", np.asarray(in_b, np.float64).reshape(NH, DH), rw)
    geoff = np.tile(np.arange(NE) * CAP, NH)
    shared = {
        "inwt": inwt,
        "cfold": np.ascontiguousarray(cfold, np.float32),
        "lb": np.ascontiguousarray(lb.reshape(NH * NE), np.float32),
        "inb": np.ascontiguousarray(in_b, np.float32),
        "outb": np.ascontiguousarray(out_b, np.float32),
        "geoff": np.ascontiguousarray(geoff, np.float32),
        "ustrict": np.ascontiguousarray(
            np.triu(np.ones((P, P), np.float32), k=1)),
        "onesm": np.ones((P, P), np.float32),
        "w_int": np.ascontiguousarray(
            np.asarray(w_in).transpose(0, 1, 3, 2)).astype(BFNP),
        "w_outt": np.ascontiguousarray(np.asarray(w_out)).astype(BFNP),
        "outwt": np.ascontiguousarray(np.asarray(out_w).T).astype(BFNP),
    }
    in_maps = []
    for c in range(NCORES):
        xs = x[c * TLOC:(c + 1) * TLOC]                              # (TLOC, D)
        in_maps.append({"xt": np.ascontiguousarray(xs.T), **shared})
    return in_maps


def kernel(**inputs) -> np.ndarray:
    global _CACHED
    if _CACHED is None:
        _CACHED = build_program()
    nc = _CACHED
    in_maps = _prep(
        np.asarray(inputs["x"]), np.asarray(inputs["in_w"]),
        np.asarray(inputs["in_b"]), np.asarray(inputs["router_w"]),
        np.asarray(inputs["w_in"]), np.asarray(inputs["w_out"]),
        np.asarray(inputs["out_w"]), np.asarray(inputs["out_b"]))
    global LAST_RESULT
    res = run_bass_kernel_spmd(
        nc, in_maps, core_ids=list(range(NCORES)), trace=TRACE)
    LAST_RESULT = res
    return np.concatenate(
        [np.ascontiguousarray(res.results[c]["out_t"].T) for c in range(NCORES)],
        axis=0)


# revision 18
# speedup vs baseline: 1.1048x; 1.1048x over previous
"""
MultiHeadLatentMoE layer as a Bass/Tile kernel for 8 Trainium2 NeuronCores.

Problem (T=8192, D=1024, NH=8 heads, DH=128, NE=8 experts/head, top-2, DHID=512):
    h      = (x @ in_w.T + in_b).reshape(T, NH, DH)
    logits = einsum('tnd,ned->tne', h, router_w)            (fp32)
    gate   = scatter(softmax(top2(logits)))                  (T, NH, NE)
    hid    = gelu(einsum('tnd,nefd->tnef', h, w_in))         (exact erf gelu)
    ye     = einsum('tnef,nefd->tned', hid, w_out)
    y      = einsum('tne,tned->tnd', gate, ye)
    out    = y.reshape(T, NH*DH) @ out_w.T + out_b

Strategy (v2, sparse): data-parallel over tokens (1024/core, zero collectives),
but unlike the dense baseline only the top-2 of 8 experts are computed:

  1. in-proj + router run in token-on-partitions layout.  Router logits come
     from a host-folded matrix C = in_w_head.T @ router_w (exact fp32), so
     routing matches the fp32 reference while h itself is a single fp32r pass.
  2. top-2 masks -> per-(head,expert) positions via a triangular-matmul cumsum
     -> global slot ids (capacity 384/expert, 9 sigma above the 256 mean).
  3. dispatch: indirect DMA scatter of h rows (bf16) into a slot-major DRAM
     table; per head the table is read back transposed (xbar DMA) so the
     expert FFNs run dense bf16 matmuls over 3072 slots instead of 8192.
  4. gates are NOT applied in the expert phase; the combine step indirect-
     gathers each token's two ye rows and blends with w1/w2 on DVE.
  5. out-proj in bf16.

All big matmuls are bf16/fp32r at 1 cycle/row; expert FLOPs drop 2.7x.
"""

import sys

for _p in ("/opt/trn_rl_repo", "/root/.axon_site/_ro/trn_rl_repo"):
    if _p not in sys.path:
        sys.path.append(_p)

import numpy as np
import ml_dtypes

import concourse.bass as bass
import concourse.mybir as mybir
import concourse.tile as tile
from concourse import bacc
from concourse.bass_utils import run_bass_kernel_spmd
from concourse.masks import make_identity

T, D, NH, DH, NE, TOPK, DHID = 8192, 1024, 8, 128, 8, 2, 512
NCORES = 8
TLOC = T // NCORES          # 1024 tokens per core
P = 128
KT = D // P                 # 8 contraction k-tiles for D=1024
NT = TLOC // P              # 8 token tiles of 128
FT = DHID // P              # 4 dhid tiles per expert
CAP = 384                   # token capacity per (head, expert); mean 256
SPH = NE * CAP              # slots per head (3072)
NSLOT = NH * SPH            # total slots (24576)
ST = CAP // P               # slot tiles per expert (3)
F32 = mybir.dt.float32
F32R = mybir.dt.float32r
BF16 = mybir.dt.bfloat16
I32 = mybir.dt.int32
BFNP = ml_dtypes.bfloat16

_CACHED = None
TRACE = False          # set True (e.g. from test.py) to neuron-profile the run
LAST_RESULT = None     # BassKernelResults of the most recent kernel() call
SIM = False            # True: full-table indirect APs (CoreSim-compatible);
                       # False: HW contract (indirect AP rows == #indices)
DEBUG = False          # True: add debug ExternalOutputs for intermediates


def build_program():
    nc = bacc.Bacc()

    xt_d = nc.dram_tensor("xt", [D, TLOC], F32R, kind="ExternalInput")
    inwt_d = nc.dram_tensor("inwt", [D, D], F32R, kind="ExternalInput")
    cfhi_d = nc.dram_tensor("cfhi", [D, NH * NE], F32R, kind="ExternalInput")
    cflo_d = nc.dram_tensor("cflo", [D, NH * NE], F32R, kind="ExternalInput")
    lb_d = nc.dram_tensor("lb", [NH * NE], F32, kind="ExternalInput")
    inb_d = nc.dram_tensor("inb", [D], F32, kind="ExternalInput")
    outb_d = nc.dram_tensor("outb", [D], F32, kind="ExternalInput")
    geoff_d = nc.dram_tensor("geoff", [NH * NE], F32, kind="ExternalInput")
    u_d = nc.dram_tensor("ustrict", [P, P], F32, kind="ExternalInput")
    ones_d = nc.dram_tensor("onesm", [P, P], F32, kind="ExternalInput")
    w_int_d = nc.dram_tensor("w_int", [NH, NE, DH, DHID], BF16, kind="ExternalInput")
    w_outt_d = nc.dram_tensor("w_outt", [NH, NE, DHID, DH], BF16, kind="ExternalInput")
    outwt_d = nc.dram_tensor("outwt", [D, D], BF16, kind="ExternalInput")
    hgt_ds = [nc.dram_tensor(f"hgt{n}", [SPH, DH], BF16) for n in range(NH)]
    yet_ds = [nc.dram_tensor(f"yet{n}", [SPH, DH], BF16) for n in range(NH)]
    out_d = nc.dram_tensor("out_t", [D, TLOC], F32, kind="ExternalOutput")
    if DEBUG:
        dbg_slot1 = nc.dram_tensor("dbg_slot1", [P, NT, NH], I32, kind="ExternalOutput")
        dbg_slot2 = nc.dram_tensor("dbg_slot2", [P, NT, NH], I32, kind="ExternalOutput")
        dbg_w1 = nc.dram_tensor("dbg_w1", [P, NT, NH], F32, kind="ExternalOutput")
        dbg_w2 = nc.dram_tensor("dbg_w2", [P, NT, NH], F32, kind="ExternalOutput")
        dbg_h = nc.dram_tensor("dbg_h", [NT, P, D], BF16, kind="ExternalOutput")
        dbg_hg = nc.dram_tensor("dbg_hg", [P, SPH], BF16, kind="ExternalOutput")
        dbg_hgt0 = nc.dram_tensor("dbg_hgt0", [SPH, DH], BF16, kind="ExternalOutput")
        dbg_yet0 = nc.dram_tensor("dbg_yet0", [SPH, DH], BF16, kind="ExternalOutput")
        dbg_y = nc.dram_tensor("dbg_y", [P, NH, TLOC], BF16, kind="ExternalOutput")

    Act = mybir.ActivationFunctionType
    Alu = mybir.AluOpType
    X = mybir.AxisListType.X

    with tile.TileContext(nc) as tc:
        with (
            tc.tile_pool(name="persist", bufs=1) as persist,
            tc.tile_pool(name="work", bufs=2) as work,
            tc.tile_pool(name="psum", bufs=1, space="PSUM") as psum,
        ):
            # ---- persistent SBUF ----
            ident = persist.tile([P, P], F32, tag="ident")
            make_identity(nc, ident)
            xt_sb = persist.tile([P, KT, TLOC], F32R, tag="xt")
            nc.sync.dma_start(xt_sb[:], xt_d[:].rearrange("(kt p) t -> p kt t", p=P))
            inwt_sb = persist.tile([P, KT, D], F32R, tag="inwt")
            nc.sync.dma_start(
                inwt_sb[:], inwt_d[:].rearrange("(kt p) d -> p kt d", p=P))
            chi_sb = persist.tile([P, KT, NH * NE], F32R, tag="cfhi")
            nc.sync.dma_start(
                chi_sb[:], cfhi_d[:].rearrange("(kt p) ne -> p kt ne", p=P))
            clo_sb = persist.tile([P, KT, NH * NE], F32R, tag="cflo")
            nc.sync.dma_start(
                clo_sb[:], cflo_d[:].rearrange("(kt p) ne -> p kt ne", p=P))
            u_sb = persist.tile([P, P], F32, tag="u")
            nc.sync.dma_start(u_sb[:], u_d[:])
            ones_sb = persist.tile([P, P], F32, tag="ones")
            nc.sync.dma_start(ones_sb[:], ones_d[:])
            inb_bc = persist.tile([P, D], F32, tag="inb")
            nc.sync.dma_start(inb_bc[:], inb_d[None, :].to_broadcast([P, D]))
            lb_bc = persist.tile([P, NH * NE], F32, tag="lb")
            nc.sync.dma_start(lb_bc[:], lb_d[None, :].to_broadcast([P, NH * NE]))
            geoff_bc = persist.tile([P, NH * NE], F32, tag="geoff")
            nc.sync.dma_start(
                geoff_bc[:], geoff_d[None, :].to_broadcast([P, NH * NE]))
            outb_sb = persist.tile([P, KT], F32, tag="outb")
            nc.sync.dma_start(outb_sb[:], outb_d[:].rearrange("(m p) -> p m", p=P))

            slot1_all = persist.tile([P, NT, NH], I32, tag="s1")
            slot2_all = persist.tile([P, NT, NH], I32, tag="s2")
            w1_all = persist.tile([P, NT, NH], F32, tag="w1")
            w2_all = persist.tile([P, NT, NH], F32, tag="w2")
            msum = persist.tile([P, NH * NE], F32, tag="msum")
            nc.vector.memset(msum[:], 0.0)
            y_sb = persist.tile([P, NH, TLOC], BF16, tag="y")

            # ============ Phase 1: in-proj + router + slots (t-par) ============
            for tk in range(NT):
                tsl = slice(tk * P, (tk + 1) * P)
                h_bf = work.tile([P, D], BF16, tag="hbf")
                for hb in range(2):
                    h_ps = psum.tile([P, 512], F32, tag="misc", bufs=2)
                    csl = slice(hb * 512, (hb + 1) * 512)
                    for kt in range(KT):
                        nc.tensor.matmul(
                            h_ps[:],
                            lhsT=xt_sb[:, kt, tsl],
                            rhs=inwt_sb[:, kt, csl],
                            start=(kt == 0),
                            stop=(kt == KT - 1),
                        )
                    # h = psum + in_b  (free-axis bias), cast to bf16
                    nc.vector.tensor_tensor(
                        h_bf[:, csl], h_ps[:], inb_bc[:, csl], Alu.add)

                # exact fp32 logits via explicit FP22 hi/lo splits: every
                # operand of every term is exactly representable in the PE's
                # FP22 read path, so routing matches the fp32 reference.
                xtmp = work.tile([P, KT, P], I32, tag="xtmp")
                nc.vector.tensor_scalar(
                    xtmp[:], xt_sb[:, :, tsl].bitcast(I32),
                    -8192, None, Alu.bitwise_and)
                xhi = work.tile([P, KT, P], F32R, tag="xhi")
                nc.vector.tensor_copy(xhi[:], xtmp[:].bitcast(F32))
                xlo = work.tile([P, KT, P], F32R, tag="xlo")
                nc.vector.tensor_sub(
                    xlo[:], xt_sb[:, :, tsl].bitcast(F32),
                    xtmp[:].bitcast(F32))
                lg_ps = psum.tile([P, NH * NE], F32, tag="misc", bufs=2)
                terms = [(xhi, chi_sb), (xhi, clo_sb), (xlo, chi_sb)]
                for i, (xv, cv) in enumerate(terms):
                    for kt in range(KT):
                        nc.tensor.matmul(
                            lg_ps[:],
                            lhsT=xv[:, kt, :],
                            rhs=cv[:, kt, :],
                            start=(i == 0 and kt == 0),
                            stop=(i == len(terms) - 1 and kt == KT - 1),
                        )
                lg = work.tile([P, NH, NE], F32, tag="lg")
                nc.vector.tensor_tensor(
                    lg[:], lg_ps[:].rearrange("p (n e) -> p n e", n=NH),
                    lb_bc[:].rearrange("p (n e) -> p n e", n=NH), Alu.add)

                # top-2 selection per head
                m1 = work.tile([P, NH], F32, tag="m1")
                nc.vector.tensor_reduce(m1[:], lg[:], X, Alu.max)
                eq1 = work.tile([P, NH, NE], F32, tag="eq1")
                nc.vector.tensor_tensor(
                    eq1[:], lg[:], m1[:, :, None].to_broadcast([P, NH, NE]),
                    Alu.is_equal)
                msk = work.tile([P, NH, NE], F32, tag="msk")
                nc.vector.scalar_tensor_tensor(
                    msk[:], eq1[:], -1e30, lg[:], Alu.mult, Alu.add)
                m2 = work.tile([P, NH], F32, tag="m2")
                nc.vector.tensor_reduce(m2[:], msk[:], X, Alu.max)
                eq2 = work.tile([P, NH, NE], F32, tag="eq2")
                nc.vector.tensor_tensor(
                    eq2[:], lg[:], m2[:, :, None].to_broadcast([P, NH, NE]),
                    Alu.is_equal)
                dm = work.tile([P, NH], F32, tag="dm")
                nc.vector.tensor_sub(dm[:], m2[:], m1[:])
                nc.scalar.activation(w2_all[:, tk, :], dm[:], Act.Sigmoid)
                nc.vector.tensor_scalar(
                    w1_all[:, tk, :], w2_all[:, tk, :], -1.0, 1.0,
                    Alu.mult, Alu.add)

                # slot ids: pos = (#selected before t) via triangular matmul
                mask = work.tile([P, NH * NE], F32, tag="mask")
                nc.vector.tensor_tensor(
                    mask[:].rearrange("p (n e) -> p n e", n=NH), eq1[:], eq2[:],
                    Alu.add)
                pos_ps = psum.tile([P, NH * NE], F32, tag="misc", bufs=2)
                nc.tensor.matmul(
                    pos_ps[:], lhsT=u_sb[:], rhs=mask[:], start=True, stop=False)
                nc.tensor.matmul(
                    pos_ps[:], lhsT=ones_sb[:], rhs=msum[:], start=False, stop=True)
                nc.vector.tensor_tensor(msum[:], msum[:], mask[:], Alu.add)
                # clamp to capacity (overflow at 9 sigma: last slot reused)
                posc = work.tile([P, NH * NE], F32, tag="posc")
                nc.vector.tensor_scalar_min(posc[:], pos_ps[:], float(CAP - 1))
                slotv = work.tile([P, NH * NE], F32, tag="slotv")
                nc.vector.tensor_tensor(slotv[:], posc[:], geoff_bc[:], Alu.add)
                for sl_all, eq in ((slot1_all, eq1), (slot2_all, eq2)):
                    t1 = work.tile([P, NH, NE], F32, tag="t1")
                    nc.vector.tensor_tensor(
                        t1[:], eq[:],
                        slotv[:].rearrange("p (n e) -> p n e", n=NH), Alu.mult)
                    s1f = work.tile([P, NH], F32, tag="s1f")
                    nc.vector.tensor_reduce(s1f[:], t1[:], X, Alu.add)
                    # exact-tie safety: a non-one-hot eq sums two slot ids;
                    # clamp so indices stay inside the per-head table
                    nc.vector.tensor_scalar_min(s1f[:], s1f[:], float(SPH - 1))
                    nc.vector.tensor_copy(sl_all[:, tk, :], s1f[:])

                # dispatch: scatter h rows (bf16) to slot-major table.
                # HW contract: indirect-side AP row count == #indices (128),
                # idx is a per-partition column; index*rowstride addresses the
                # full table regardless of the AP slice.
                if DEBUG:
                    nc.sync.dma_start(dbg_h[tk], h_bf[:])
                for sl_all in (slot1_all, slot2_all):
                    for n in range(NH):
                        hrows = hgt_ds[n][:] if SIM else hgt_ds[n][:P, :]
                        nc.gpsimd.indirect_dma_start(
                            out=hrows,
                            out_offset=bass.IndirectOffsetOnAxis(
                                ap=sl_all[:, tk, n:n + 1], axis=0),
                            in_=h_bf[:, n * DH:(n + 1) * DH],
                            in_offset=None,
                        )

            if DEBUG:
                nc.sync.dma_start(dbg_slot1[:], slot1_all[:])
                nc.sync.dma_start(dbg_slot2[:], slot2_all[:])
                nc.sync.dma_start(dbg_w1[:], w1_all[:])
                nc.sync.dma_start(dbg_w2[:], w2_all[:])

            # ============ Phase 2: experts (sparse slots, bf16) ============
            with tc.tile_pool(name="hpool", bufs=2) as hpool, \
                 tc.tile_pool(name="epool", bufs=3) as epool:
                for n in range(NH):
                    hg = hpool.tile([P, SPH], BF16, tag="hg")
                    nc.sync.dma_start_transpose(hg[:], hgt_ds[n][:])
                    # whole head's expert outputs staged in SBUF, written to
                    # DRAM as ONE full-tensor DMA: the combine gather's
                    # nominal read region (rows 0..127) is then strictly
                    # inside this write region, giving correct ordering.
                    yet_all = hpool.tile([P, NE * ST, DH], BF16, tag="yetall")
                    if DEBUG and n == 0:
                        nc.sync.dma_start(dbg_hg[:], hg[:])
                        nc.sync.dma_start(dbg_hgt0[:], hgt_ds[0][:])
                    for e in range(NE):
                        wi = epool.tile([P, DHID], BF16, tag="wi")
                        nc.sync.dma_start(wi[:], w_int_d[n, e])
                        wo = epool.tile([P, FT, DH], BF16, tag="wo")
                        nc.sync.dma_start(
                            wo[:],
                            w_outt_d[n, e].rearrange("(f p) d -> p f d", p=P))
                        esl = slice(e * CAP, (e + 1) * CAP)
                        hidg = epool.tile([P, FT, CAP], BF16, tag="hidg")
                        for f in range(FT):
                            hid_ps = psum.tile(
                                [P, 512], F32, tag=f"hid{f}", bufs=1)
                            nc.tensor.matmul(
                                hid_ps[:, :CAP],
                                lhsT=wi[:, f * P:(f + 1) * P],
                                rhs=hg[:, esl],
                                start=True, stop=True,
                            )
                            nc.scalar.activation(
                                hidg[:, f, :], hid_ps[:, :CAP], Act.Gelu)
                        for st in range(ST):
                            yet_ps = psum.tile([P, DH], F32, tag="misc", bufs=2)
                            ssl = slice(st * P, (st + 1) * P)
                            for f in range(FT):
                                nc.tensor.matmul(
                                    yet_ps[:],
                                    lhsT=hidg[:, f, ssl],
                                    rhs=wo[:, f, :],
                                    start=(f == 0),
                                    stop=(f == FT - 1),
                                )
                            nc.vector.tensor_copy(
                                yet_all[:, e * ST + st, :], yet_ps[:])
                    nc.sync.dma_start(
                        yet_ds[n][:].rearrange("(j p) d -> p j d", p=P),
                        yet_all[:])

            if DEBUG:
                nc.sync.dma_start(dbg_yet0[:], yet_ds[0][:])

            # ============ Phase 3: combine (gather + blend + transpose) ========
            for tk in range(NT):
                tsl = slice(tk * P, (tk + 1) * P)
                y1 = work.tile([P, NH, DH], BF16, tag="y1")
                y2 = work.tile([P, NH, DH], BF16, tag="y2")
                for yt, sl_all in ((y1, slot1_all), (y2, slot2_all)):
                    for n in range(NH):
                        yrows = yet_ds[n][:] if SIM else yet_ds[n][:P, :]
                        nc.gpsimd.indirect_dma_start(
                            out=yt[:, n, :],
                            out_offset=None,
                            in_=yrows,
                            in_offset=bass.IndirectOffsetOnAxis(
                                ap=sl_all[:, tk, n:n + 1], axis=0),
                        )
                ya = work.tile([P, NH, DH], F32, tag="ya")
                nc.vector.tensor_tensor(
                    ya[:], y1[:],
                    w1_all[:, tk, :, None].to_broadcast([P, NH, DH]), Alu.mult)
                yb = work.tile([P, NH, DH], F32, tag="yb")
                nc.vector.tensor_tensor(
                    yb[:], y2[:],
                    w2_all[:, tk, :, None].to_broadcast([P, NH, DH]), Alu.mult)
                yc = work.tile([P, NH, DH], F32, tag="yc")
                nc.vector.tensor_tensor(yc[:], ya[:], yb[:], Alu.add)
                for n in range(NH):
                    tp_ps = psum.tile([P, P], F32, tag="misc", bufs=2)
                    nc.tensor.transpose(tp_ps[:], yc[:, n, :], ident[:])
                    nc.vector.tensor_copy(y_sb[:, n, tsl], tp_ps[:])

            if DEBUG:
                nc.sync.dma_start(dbg_y[:], y_sb[:])

            # ============ Phase 4: out-projection (bf16) ============
            with tc.tile_pool(name="opool", bufs=2) as opool:
                for m in range(KT):
                    ow = opool.tile([P, KT, P], BF16, tag="ow")
                    nc.sync.dma_start(
                        ow[:],
                        outwt_d[:, m * P:(m + 1) * P].rearrange(
                            "(kt p) d -> p kt d", p=P))
                    o_sb = opool.tile([P, TLOC], F32, tag="osb")
                    for tt in range(2):
                        tsl = slice(tt * 512, (tt + 1) * 512)
                        o_ps = psum.tile([P, 512], F32, tag="misc", bufs=2)
                        for kt in range(KT):
                            nc.tensor.matmul(
                                o_ps[:],
                                lhsT=ow[:, kt, :],
                                rhs=y_sb[:, kt, tsl],
                                start=(kt == 0),
                                stop=(kt == KT - 1),
                            )
                        nc.scalar.activation(
                            o_sb[:, tsl], o_ps[:], Act.Identity,
                            bias=outb_sb[:, m:m + 1])
                    nc.sync.dma_start(out_d[m * P:(m + 1) * P, :], o_sb[:])

    nc.compile()
    return nc


def _prep(x, in_w, in_b, router_w, w_in, w_out, out_w, out_b):
    """Host-side layout prep: transposes, bf16 casts, folded router."""
    x = np.ascontiguousarray(x, dtype=np.float32)
    inwt = np.ascontiguousarray(in_w.T, dtype=np.float32)            # (D, D)
    iw = np.asarray(in_w, np.float64).reshape(NH, DH, D)
    rw = np.asarray(router_w, np.float64)                            # (NH,NE,DH)
    cfold = np.ascontiguousarray(
        np.einsum("nhd,neh->dne", iw, rw).reshape(D, NH * NE), np.float32)
    cfhi = (cfold.view(np.uint32) & np.uint32(0xFFFFE000)).view(np.float32)
    cflo = np.ascontiguousarray(cfold - cfhi)
    lb = np.einsum(
        "nh,neh->ne", np.asarray(in_b, np.float64).reshape(NH, DH), rw)
    geoff = np.tile(np.arange(NE) * CAP, NH)
    shared = {
        "inwt": inwt,
        "cfhi": cfhi,
        "cflo": cflo,
        "lb": np.ascontiguousarray(lb.reshape(NH * NE), np.float32),
        "inb": np.ascontiguousarray(in_b, np.float32),
        "outb": np.ascontiguousarray(out_b, np.float32),
        "geoff": np.ascontiguousarray(geoff, np.float32),
        "ustrict": np.ascontiguousarray(
            np.triu(np.ones((P, P), np.float32), k=1)),
        "onesm": np.ones((P, P), np.float32),
        "w_int": np.ascontiguousarray(
            np.asarray(w_in).transpose(0, 1, 3, 2)).astype(BFNP),
        "w_outt": np.ascontiguousarray(np.asarray(w_out)).astype(BFNP),
        "outwt": np.ascontiguousarray(np.asarray(out_w).T).astype(BFNP),
    }
    in_maps = []
    for c in range(NCORES):
        xs = x[c * TLOC:(c + 1) * TLOC]                              # (TLOC, D)
        in_maps.append({"xt": np.ascontiguousarray(xs.T), **shared})
    return in_maps


def kernel(**inputs) -> np.ndarray:
    global _CACHED
    if _CACHED is None:
        _CACHED = build_program()
    nc = _CACHED
    in_maps = _prep(
        np.asarray(inputs["x"]), np.asarray(inputs["in_w"]),
        np.asarray(inputs["in_b"]), np.asarray(inputs["router_w"]),
        np.asarray(inputs["w_in"]), np.asarray(inputs["w_out"]),
        np.asarray(inputs["out_w"]), np.asarray(inputs["out_b"]))
    global LAST_RESULT
    res = run_bass_kernel_spmd(
        nc, in_maps, core_ids=list(range(NCORES)), trace=TRACE)
    LAST_RESULT = res
    return np.concatenate(
        [np.ascontiguousarray(res.results[c]["out_t"].T) for c in range(NCORES)],
        axis=0)
